# revision 9
# baseline (speedup 1.0000x reference)
"""Trainium2 Bass kernel for an 8-expert top-2 MoE (nn_BaseMoe).

Strategy (data-parallel routed, 2 launches, all FLOPs on device):
  - 8 cores; core c owns tokens [c*2048, (c+1)*2048).
  - Launch 1 (router): per core, logits = x_shard @ gate_w.T (fp32 PE matmul)
    and probs = softmax(logits) on device. Host only does index bookkeeping:
    stable top-2 selection, per-(core, expert) token buckets, bucket
    positions for the final combine.
  - Launch 2 (MoE): per core, for each expert e: indirect-DMA gather the
    bucket's token rows, SwiGLU MLP in float32r (1 cyc/row on PE), write
    rows to an internal DRAM buffer Y. Finally, per owned token, gather its
    two expert rows from Y and do the weighted sum on device.

The kernel returns (out [4,4096,2048] f32, router_logits [16384,8] f32),
matching the reference's tuple.
"""

import sys
import functools

sys.path.insert(0, "/opt/trn_rl_repo")

import numpy as np

import concourse.bass as bass
import concourse.mybir as mybir
from concourse import bacc
from concourse.tile import TileContext
from concourse.bass_utils import run_bass_kernel_spmd
from concourse.masks import make_identity

F32 = mybir.dt.float32
F32R = mybir.dt.float32r
I32 = mybir.dt.int32
AX = mybir.AxisListType
ALU = mybir.AluOpType
ACT = mybir.ActivationFunctionType

B, S, H, I, E, TOPK = 4, 4096, 2048, 1024, 8, 2
T = B * S            # 16384 tokens
NCORES = 8
TPC = T // NCORES    # 2048 tokens per core
P = 128              # partitions
HK = H // P          # 16 contraction chunks over H
IK = I // P          # 8 contraction chunks over I
NQ = 4               # I split into 4 quarters for gate/up weight streaming
IQ = I // NQ         # 256

CORE_IDS = list(range(NCORES))
_LAST_CAPS = None


# --------------------------------------------------------------------------
# Launch 1: router
# --------------------------------------------------------------------------
@functools.lru_cache(maxsize=1)
def _router_nc():
    nc = bacc.Bacc()
    xs_d = nc.declare_dram_parameter("xs", [TPC, H], F32, isOutput=False)
    gwt_d = nc.declare_dram_parameter("gwt", [H, E], F32, isOutput=False)
    logits_d = nc.declare_dram_parameter("logits", [TPC, E], F32, isOutput=True)
    probs_d = nc.declare_dram_parameter("probs", [TPC, E], F32, isOutput=True)

    with TileContext(nc) as tc:
        with tc.tile_pool(name="const", bufs=1) as cpool, \
             tc.tile_pool(name="sb", bufs=3) as sb, \
             tc.tile_pool(name="ps", bufs=2, space="PSUM") as ps:
            ident = cpool.tile([P, P], F32)
            make_identity(nc, ident[:])
            gwt_sb = cpool.tile([P, HK, E], F32)
            nc.sync.dma_start(
                out=gwt_sb[:], in_=gwt_d[:].rearrange("(k p) e -> p k e", p=P))

            for ti in range(TPC // P):
                xg = sb.tile([P, H], F32, tag="xg")
                nc.sync.dma_start(out=xg[:], in_=xs_d[ti * P:(ti + 1) * P, :])
                lg_ps = ps.tile([P, E], F32, tag="lg")
                for k in range(HK):
                    tp = ps.tile([P, P], F32, tag="tp")
                    nc.tensor.transpose(
                        out=tp[:], in_=xg[:, k * P:(k + 1) * P], identity=ident[:])
                    xtk = sb.tile([P, P], F32, tag="xtk")
                    nc.vector.tensor_copy(out=xtk[:], in_=tp[:])
                    nc.tensor.matmul(out=lg_ps[:], lhsT=xtk[:], rhs=gwt_sb[:, k, :],
                                     start=(k == 0), stop=(k == HK - 1))
                lg = sb.tile([P, E], F32, tag="lgsb")
                nc.vector.tensor_copy(out=lg[:], in_=lg_ps[:])
                nc.sync.dma_start(out=logits_d[ti * P:(ti + 1) * P, :], in_=lg[:])

                # softmax over the 8 experts (free dim)
                rmax = sb.tile([P, 1], F32, tag="rmax")
                nc.vector.reduce_max(rmax[:], lg[:], axis=AX.X)
                negmax = sb.tile([P, 1], F32, tag="negmax")
                nc.vector.tensor_scalar_mul(negmax[:], rmax[:], -1.0)
                pex = sb.tile([P, E], F32, tag="pex")
                nc.scalar.activation(out=pex[:], in_=lg[:], func=ACT.Exp,
                                     bias=negmax[:, :1], scale=1.0)
                ssum = sb.tile([P, 1], F32, tag="ssum")
                nc.vector.reduce_sum(ssum[:], pex[:], axis=AX.X)
                rs = sb.tile([P, 1], F32, tag="rs")
                nc.vector.reciprocal(rs[:], ssum[:])
                probs = sb.tile([P, E], F32, tag="probs")
                nc.vector.tensor_scalar_mul(probs[:], pex[:], rs[:, :1])
                nc.sync.dma_start(out=probs_d[ti * P:(ti + 1) * P, :], in_=probs[:])

    nc.finalize()
    return nc


# --------------------------------------------------------------------------
# Launch 2: bucketed expert MLP + on-device combine
# --------------------------------------------------------------------------
@functools.lru_cache(maxsize=4)
def _moe_nc(caps):
    """caps: tuple of 8 per-expert bucket capacities (multiples of 128)."""
    caps = tuple(int(c) for c in caps)
    nb = sum(caps)
    bases = np.cumsum([0] + list(caps))[:-1]

    nc = bacc.Bacc()
    xs_d = nc.declare_dram_parameter("xs", [TPC, H], F32R, isOutput=False)
    wg_d = nc.declare_dram_parameter("wg", [E, NQ, P, HK, IQ], F32R, isOutput=False)
    wu_d = nc.declare_dram_parameter("wu", [E, NQ, P, HK, IQ], F32R, isOutput=False)
    wd_d = nc.declare_dram_parameter("wd", [E, 4, P, IK, 512], F32R, isOutput=False)
    bidx_d = nc.declare_dram_parameter("bidx", [nb], I32, isOutput=False)
    posw_d = nc.declare_dram_parameter("posw", [TPC, 2], I32, isOutput=False)
    ww_d = nc.declare_dram_parameter("ww", [TPC, 2], F32, isOutput=False)
    y_d = nc.declare_dram_parameter("y", [TPC, H], F32, isOutput=True)
    Y_d = nc.dram_tensor("Ybuf", [nb, H], F32)

    bidx_t = bidx_d[:].rearrange("(n p o) -> n p o", p=P, o=1)

    with TileContext(nc) as tc:
        with tc.tile_pool(name="const", bufs=1) as cpool, \
             tc.tile_pool(name="wab", bufs=2) as wab, \
             tc.tile_pool(name="wdp", bufs=2) as wdp, \
             tc.tile_pool(name="xt", bufs=1) as xtp, \
             tc.tile_pool(name="hb", bufs=1) as hbp, \
             tc.tile_pool(name="sb", bufs=2) as sb, \
             tc.tile_pool(name="sm", bufs=2) as sm, \
             tc.tile_pool(name="psA", bufs=2, space="PSUM") as psA, \
             tc.tile_pool(name="pstp", bufs=2, space="PSUM") as pstp, \
             tc.tile_pool(name="psY", bufs=2, space="PSUM") as psY:
            ident = sm.tile([P, P], F32, tag="small")
            make_identity(nc, ident[:])
            ident_r = cpool.tile([P, P], F32R)
            nc.vector.tensor_copy(out=ident_r[:], in_=ident[:])

            for e in range(E):
                ntiles = caps[e] // P
                xts = []
                hts = []
                # gather + transpose this expert's bucket rows
                for t in range(ntiles):
                    flat = (bases[e] + t * P) // P
                    idx = sm.tile([P, 1], I32, tag="small", name=f"idx{e}_{t}")
                    nc.sync.dma_start(out=idx[:], in_=bidx_t[flat])
                    xg = sb.tile([P, H], F32R, tag="xg", name=f"xg{e}_{t}")
                    nc.gpsimd.indirect_dma_start(
                        out=xg[:], out_offset=None, in_=xs_d[:],
                        in_offset=bass.IndirectOffsetOnAxis(ap=idx[:, :1], axis=0))
                    xt = xtp.tile([P, HK, P], F32R, tag=f"xt{t}", name=f"xt{t}_{e}")
                    for k in range(HK):
                        tp = pstp.tile([P, P], F32R, tag="tp", name=f"tpx{e}_{t}_{k}")
                        nc.tensor.transpose(
                            out=tp[:], in_=xg[:, k * P:(k + 1) * P],
                            identity=ident_r[:])
                        nc.vector.tensor_copy(out=xt[:, k, :], in_=tp[:])
                    xts.append(xt)
                    ht = hbp.tile([P, IK, P], F32R, tag=f"h{t}", name=f"h{t}_{e}")
                    hts.append(ht)

                # pass A: gate/up in I quarters (weights streamed, double-buffered)
                # h is stored pre-transposed: hts[t][:, c, :] = h[:, c*128:+128].T
                for q in range(NQ):
                    qsl = slice(q * IQ, (q + 1) * IQ)
                    wgq = wab.tile([P, HK, IQ], F32R, tag="wgq", name=f"wgq{e}_{q}")
                    nc.sync.dma_start(out=wgq[:], in_=wg_d[e, q])
                    wuq = wab.tile([P, HK, IQ], F32R, tag="wuq", name=f"wuq{e}_{q}")
                    nc.sync.dma_start(out=wuq[:], in_=wu_d[e, q])
                    for t in range(ntiles):
                        hg = psA.tile([P, IQ], F32, tag="hg", name=f"hg{e}_{q}_{t}")
                        hu = psA.tile([P, IQ], F32, tag="hu", name=f"hu{e}_{q}_{t}")
                        for k in range(HK):
                            nc.tensor.matmul(out=hg[:], lhsT=xts[t][:, k, :],
                                             rhs=wgq[:, k, :],
                                             start=(k == 0), stop=(k == HK - 1))
                            nc.tensor.matmul(out=hu[:], lhsT=xts[t][:, k, :],
                                             rhs=wuq[:, k, :],
                                             start=(k == 0), stop=(k == HK - 1))
                        sg = sm.tile([P, IQ], F32, tag="small", name=f"sg{e}_{q}_{t}")
                        nc.scalar.activation(out=sg[:], in_=hg[:], func=ACT.Silu)
                        hq = sm.tile([P, IQ], F32R, tag="small", name=f"hq{e}_{q}_{t}")
                        nc.vector.tensor_tensor(out=hq[:], in0=sg[:],
                                                in1=hu[:], op=ALU.mult)
                        # transpose the two 128-chunks of this quarter into hts
                        for j in range(IQ // P):
                            tp = pstp.tile([P, P], F32R, tag="tp",
                                           name=f"tph{e}_{q}_{t}_{j}")
                            nc.tensor.transpose(out=tp[:],
                                                in_=hq[:, j * P:(j + 1) * P],
                                                identity=ident_r[:])
                            nc.vector.tensor_copy(
                                out=hts[t][:, q * (IQ // P) + j, :], in_=tp[:])

                # pass B: down-projection; wd streamed in 512-wide column chunks
                for qc in range(4):
                    csl = slice(qc * 512, (qc + 1) * 512)  # Y column slice
                    wdt = wdp.tile([P, IK, 512], F32R, tag="wd", name=f"wd{e}_{qc}")
                    nc.sync.dma_start(out=wdt[:], in_=wd_d[e, qc])
                    for t in range(ntiles):
                        yps = psY.tile([P, 512], F32, tag="y", name=f"y{e}_{qc}_{t}")
                        for k in range(IK):
                            nc.tensor.matmul(
                                out=yps[:], lhsT=hts[t][:, k, :],
                                rhs=wdt[:, k, :],
                                start=(k == 0), stop=(k == IK - 1))
                        ysb = sm.tile([P, 512], F32, tag="small",
                                      name=f"ysb{e}_{qc}_{t}")
                        nc.vector.tensor_copy(out=ysb[:], in_=yps[:])
                        row0 = bases[e] + t * P
                        nc.sync.dma_start(out=Y_d[row0:row0 + P, csl], in_=ysb[:])

            # combine: per owned token, weighted sum of its two expert rows
            posw_t = posw_d[:].rearrange("(n p) o -> n p o", p=P)
            ww_t = ww_d[:].rearrange("(n p) o -> n p o", p=P)
            for ct in range(TPC // P):
                pp = sm.tile([P, 2], I32, tag="small", name=f"pp{ct}")
                nc.sync.dma_start(out=pp[:], in_=posw_t[ct])
                wtw = sm.tile([P, 2], F32, tag="small", name=f"wtw{ct}")
                nc.sync.dma_start(out=wtw[:], in_=ww_t[ct])
                ya = xtp.tile([P, H], F32, tag="xt0", name=f"ya{ct}")
                nc.gpsimd.indirect_dma_start(
                    out=ya[:], out_offset=None, in_=Y_d[:],
                    in_offset=bass.IndirectOffsetOnAxis(ap=pp[:, 0:1], axis=0))
                yb = xtp.tile([P, H], F32, tag="xt1", name=f"yb{ct}")
                nc.gpsimd.indirect_dma_start(
                    out=yb[:], out_offset=None, in_=Y_d[:],
                    in_offset=bass.IndirectOffsetOnAxis(ap=pp[:, 1:2], axis=0))
                tmpa = xtp.tile([P, H], F32, tag="xt2", name=f"tmpa{ct}")
                nc.vector.tensor_scalar_mul(tmpa[:], ya[:], wtw[:, 0:1])
                acc = xtp.tile([P, H], F32, tag="xt3", name=f"acc{ct}")
                nc.vector.scalar_tensor_tensor(
                    out=acc[:], in0=yb[:], scalar=wtw[:, 1:2], in1=tmpa[:],
                    op0=ALU.mult, op1=ALU.add)
                nc.sync.dma_start(out=y_d[ct * P:(ct + 1) * P, :], in_=acc[:])

    nc.finalize()
    return nc


# --------------------------------------------------------------------------
# Host orchestration
# --------------------------------------------------------------------------
def _route_host(probs):
    """Index bookkeeping only. probs: [T, 8] f32 from device.

    Returns per-core dicts of int32/f32 side inputs plus the cap tuple."""
    per_core = []
    cnt = np.zeros((NCORES, E), np.int64)
    sel_all = []
    for c in range(NCORES):
        pl = probs[c * TPC:(c + 1) * TPC]
        # stable argsort of -p == lax.top_k tie semantics (lowest index first)
        top2 = np.argsort(-pl, axis=1, kind="stable")[:, :2]
        sel_all.append(top2)
        for e in range(E):
            cnt[c, e] = int(((top2 == e).any(axis=1)).sum())
    caps = tuple(int(P * np.ceil(cnt[:, e].max() / P)) for e in range(E))
    bases = np.cumsum([0] + list(caps))[:-1]
    nb = int(sum(caps))

    for c in range(NCORES):
        pl = probs[c * TPC:(c + 1) * TPC]
        top2 = sel_all[c]
        bidx = np.zeros(nb, np.int32)
        posmap = np.zeros((TPC, E), np.int64)  # position of token t in bucket e
        for e in range(E):
            rows = np.nonzero((top2 == e).any(axis=1))[0]
            bidx[bases[e]:bases[e] + len(rows)] = rows.astype(np.int32)
            posmap[rows, e] = bases[e] + np.arange(len(rows))
        tok = np.arange(TPC)
        posw = np.stack([posmap[tok, top2[:, 0]], posmap[tok, top2[:, 1]]],
                        axis=1).astype(np.int32)
        ww = np.stack([pl[tok, top2[:, 0]], pl[tok, top2[:, 1]]],
                      axis=1).astype(np.float32)
        per_core.append({"bidx": bidx, "posw": np.ascontiguousarray(posw),
                         "ww": np.ascontiguousarray(ww)})
    return per_core, caps


def kernel(x, gate_w, wg, wu, wd):
    x = np.ascontiguousarray(x, dtype=np.float32)
    gate_w = np.ascontiguousarray(gate_w, dtype=np.float32)
    wg = np.ascontiguousarray(wg, dtype=np.float32)
    wu = np.ascontiguousarray(wu, dtype=np.float32)
    wd = np.ascontiguousarray(wd, dtype=np.float32)

    xt = x.reshape(T, H)
    gwt = np.ascontiguousarray(gate_w.T)

    # launch 1: router
    r_nc = _router_nc()
    in_maps1 = [{"xs": xt[c * TPC:(c + 1) * TPC], "gwt": gwt} for c in CORE_IDS]
    res1 = run_bass_kernel_spmd(r_nc, in_maps1, CORE_IDS).results
    logits = np.concatenate([res1[c]["logits"] for c in CORE_IDS], axis=0)
    probs = np.concatenate([res1[c]["probs"] for c in CORE_IDS], axis=0)

    # host: index bookkeeping
    side, caps = _route_host(probs)
    global _LAST_CAPS
    _LAST_CAPS = caps

    # launch 2: MoE (weights pre-staged into per-partition-contiguous layout)
    wgr = np.ascontiguousarray(
        wg.reshape(E, HK, P, NQ, IQ).transpose(0, 3, 2, 1, 4))
    wur = np.ascontiguousarray(
        wu.reshape(E, HK, P, NQ, IQ).transpose(0, 3, 2, 1, 4))
    wdr = np.ascontiguousarray(
        wd.reshape(E, IK, P, 4, 512).transpose(0, 3, 2, 1, 4))
    m_nc = _moe_nc(caps)
    in_maps2 = []
    for c in CORE_IDS:
        m = {"xs": xt[c * TPC:(c + 1) * TPC], "wg": wgr, "wu": wur, "wd": wdr}
        m.update(side[c])
        in_maps2.append(m)
    res2 = run_bass_kernel_spmd(m_nc, in_maps2, CORE_IDS).results
    out = np.concatenate([res2[c]["y"] for c in CORE_IDS], axis=0)
    return out.reshape(B, S, H), logits


# revision 32
# speedup vs baseline: 1.3059x; 1.3059x over previous
"""Trainium2 Bass kernel for an 8-expert top-2 MoE (nn_BaseMoe).

Strategy (data-parallel routed, 2 launches, all FLOPs on device):
  - 8 cores; core c owns tokens [c*2048, (c+1)*2048).
  - Launch 1 (router): per core, logits = x_shard @ gate_w.T (fp32 PE matmul)
    and probs = softmax(logits) on device. Host only does index bookkeeping:
    stable top-2 selection, per-(core, expert) token buckets, bucket
    positions for the final combine.
  - Launch 2 (MoE): per core, for each expert e: indirect-DMA gather the
    bucket's token rows, SwiGLU MLP in float32r (1 cyc/row on PE), write
    rows to an internal DRAM buffer Y. Finally, per owned token, gather its
    two expert rows from Y and do the weighted sum on device.

The kernel returns (out [4,4096,2048] f32, router_logits [16384,8] f32),
matching the reference's tuple.
"""

import sys
import functools

sys.path.insert(0, "/opt/trn_rl_repo")

import numpy as np

import concourse.bass as bass
import concourse.mybir as mybir
from concourse import bacc
from concourse.tile import TileContext
from concourse.bass_utils import run_bass_kernel_spmd
from concourse.masks import make_identity

F32 = mybir.dt.float32
F32R = mybir.dt.float32r
I32 = mybir.dt.int32
AX = mybir.AxisListType
ALU = mybir.AluOpType
ACT = mybir.ActivationFunctionType

B, S, H, I, E, TOPK = 4, 4096, 2048, 1024, 8, 2
T = B * S            # 16384 tokens
NCORES = 8
TPC = T // NCORES    # 2048 tokens per core
P = 128              # partitions
HK = H // P          # 16 contraction chunks over H
IK = I // P          # 8 contraction chunks over I
NQ = 4               # I split into 4 quarters for gate/up weight streaming
IQ = I // NQ         # 256

CORE_IDS = list(range(NCORES))
_LAST_CAPS = None


# --------------------------------------------------------------------------
# Launch 1: router
# --------------------------------------------------------------------------
@functools.lru_cache(maxsize=1)
def _router_nc():
    nc = bacc.Bacc()
    xs_d = nc.declare_dram_parameter("xs", [TPC, H], F32, isOutput=False)
    gwt_d = nc.declare_dram_parameter("gwt", [H, E], F32, isOutput=False)
    logits_d = nc.declare_dram_parameter("logits", [TPC, E], F32, isOutput=True)
    probs_d = nc.declare_dram_parameter("probs", [TPC, E], F32, isOutput=True)

    with TileContext(nc) as tc:
        with tc.tile_pool(name="const", bufs=1) as cpool, \
             tc.tile_pool(name="sb", bufs=3) as sb, \
             tc.tile_pool(name="ps", bufs=2, space="PSUM") as ps:
            ident = cpool.tile([P, P], F32)
            make_identity(nc, ident[:])
            gwt_sb = cpool.tile([P, HK, E], F32)
            nc.sync.dma_start(
                out=gwt_sb[:], in_=gwt_d[:].rearrange("(k p) e -> p k e", p=P))

            for ti in range(TPC // P):
                xg = sb.tile([P, H], F32, tag="xg")
                nc.sync.dma_start(out=xg[:], in_=xs_d[ti * P:(ti + 1) * P, :])
                lg_ps = ps.tile([P, E], F32, tag="lg")
                for k in range(HK):
                    tp = ps.tile([P, P], F32, tag="tp")
                    nc.tensor.transpose(
                        out=tp[:], in_=xg[:, k * P:(k + 1) * P], identity=ident[:])
                    xtk = sb.tile([P, P], F32, tag="xtk")
                    nc.vector.tensor_copy(out=xtk[:], in_=tp[:])
                    nc.tensor.matmul(out=lg_ps[:], lhsT=xtk[:], rhs=gwt_sb[:, k, :],
                                     start=(k == 0), stop=(k == HK - 1))
                lg = sb.tile([P, E], F32, tag="lgsb")
                nc.vector.tensor_copy(out=lg[:], in_=lg_ps[:])
                nc.sync.dma_start(out=logits_d[ti * P:(ti + 1) * P, :], in_=lg[:])

                # softmax over the 8 experts (free dim)
                rmax = sb.tile([P, 1], F32, tag="rmax")
                nc.vector.reduce_max(rmax[:], lg[:], axis=AX.X)
                negmax = sb.tile([P, 1], F32, tag="negmax")
                nc.vector.tensor_scalar_mul(negmax[:], rmax[:], -1.0)
                pex = sb.tile([P, E], F32, tag="pex")
                nc.scalar.activation(out=pex[:], in_=lg[:], func=ACT.Exp,
                                     bias=negmax[:, :1], scale=1.0)
                ssum = sb.tile([P, 1], F32, tag="ssum")
                nc.vector.reduce_sum(ssum[:], pex[:], axis=AX.X)
                rs = sb.tile([P, 1], F32, tag="rs")
                nc.vector.reciprocal(rs[:], ssum[:])
                probs = sb.tile([P, E], F32, tag="probs")
                nc.vector.tensor_scalar_mul(probs[:], pex[:], rs[:, :1])
                nc.sync.dma_start(out=probs_d[ti * P:(ti + 1) * P, :], in_=probs[:])

    nc.finalize()
    return nc


# --------------------------------------------------------------------------
# Launch 2: bucketed expert MLP + on-device combine
# --------------------------------------------------------------------------
@functools.lru_cache(maxsize=4)
def _moe_nc(caps):
    """caps: tuple of 8 per-expert bucket capacities (multiples of 128)."""
    caps = tuple(int(c) for c in caps)
    nb = sum(caps)
    bases = np.cumsum([0] + list(caps))[:-1]

    nc = bacc.Bacc()
    xs_d = nc.declare_dram_parameter("xs", [TPC, H], F32R, isOutput=False)
    wgu_d = nc.declare_dram_parameter("wgu", [E, NQ, P, HK, 2 * IQ], F32R,
                                      isOutput=False)
    wd_d = nc.declare_dram_parameter("wd", [E, 4, P, IK, 512], F32R, isOutput=False)
    bidx_d = nc.declare_dram_parameter("bidx", [nb], I32, isOutput=False)
    posw_d = nc.declare_dram_parameter("posw", [TPC, 2], I32, isOutput=False)
    ww_d = nc.declare_dram_parameter("ww", [TPC, 2], F32, isOutput=False)
    y_d = nc.declare_dram_parameter("y", [TPC, H], F32, isOutput=True)
    Y_d = nc.dram_tensor("Ybuf", [nb, H], F32)

    nflat = nb // P

    with TileContext(nc) as tc:
        with tc.tile_pool(name="const", bufs=1) as cpool, \
             tc.tile_pool(name="wab", bufs=2) as wab, \
             tc.tile_pool(name="wdp", bufs=2) as wdp, \
             tc.tile_pool(name="xt", bufs=1) as xtp, \
             tc.tile_pool(name="hb", bufs=1) as hbp, \
             tc.tile_pool(name="sb", bufs=2) as sb, \
             tc.tile_pool(name="sm", bufs=2) as sm, \
             tc.tile_pool(name="psA", bufs=3, space="PSUM") as psA, \
             tc.tile_pool(name="pstp", bufs=3, space="PSUM") as pstp, \
             tc.tile_pool(name="psY", bufs=2, space="PSUM") as psY:
            ident = sm.tile([P, P], F32, tag="small")
            make_identity(nc, ident[:])
            ident_r = cpool.tile([P, P], F32R)
            nc.vector.tensor_copy(out=ident_r[:], in_=ident[:])
            idx_all = cpool.tile([P, nflat], I32)
            nc.sync.dma_start(
                out=idx_all[:], in_=bidx_d[:].rearrange("(n p) -> p n", p=P))
            posw_all = cpool.tile([P, TPC // P, 2], I32)
            nc.sync.dma_start(
                out=posw_all[:],
                in_=posw_d[:].rearrange("(n p) o -> p n o", p=P))
            ww_all = cpool.tile([P, TPC // P, 2], F32)
            nc.sync.dma_start(
                out=ww_all[:], in_=ww_d[:].rearrange("(n p) o -> p n o", p=P))

            for e in range(E):
                ntiles = caps[e] // P
                xts = []
                hts = []
                # gather + transpose this expert's bucket rows
                for t in range(ntiles):
                    flat = (bases[e] + t * P) // P
                    xg = sb.tile([P, H], F32R, tag="xg", name=f"xg{e}_{t}")
                    nc.gpsimd.indirect_dma_start(
                        out=xg[:], out_offset=None, in_=xs_d[:],
                        in_offset=bass.IndirectOffsetOnAxis(
                            ap=idx_all[:, flat:flat + 1], axis=0))
                    xt = xtp.tile([P, HK, P], F32R, tag=f"xt{t}", name=f"xt{t}_{e}")
                    for k in range(HK):
                        tp = pstp.tile([P, P], F32R, tag="tp", name=f"tpx{e}_{t}_{k}")
                        nc.tensor.transpose(
                            out=tp[:], in_=xg[:, k * P:(k + 1) * P],
                            identity=ident_r[:])
                        nc.vector.tensor_copy(out=xt[:, k, :], in_=tp[:])
                    xts.append(xt)
                    ht = hbp.tile([P, IK, P], F32R, tag=f"h{t}", name=f"h{t}_{e}")
                    hts.append(ht)

                # pass A: gate/up in I quarters (weights streamed, double-buffered)
                # h is stored pre-transposed: hts[t][:, c, :] = h[:, c*128:+128].T
                for q in range(NQ):
                    qsl = slice(q * IQ, (q + 1) * IQ)
                    wgq = wab.tile([P, HK, 2 * IQ], F32R, tag="wgq",
                                   name=f"wgq{e}_{q}")
                    nc.sync.dma_start(out=wgq[:], in_=wgu_d[e, q])
                    for t in range(ntiles):
                        hgu = psA.tile([P, 2 * IQ], F32, tag="hgu",
                                       name=f"hgu{e}_{q}_{t}")
                        for k in range(HK):
                            nc.tensor.matmul(out=hgu[:], lhsT=xts[t][:, k, :],
                                             rhs=wgq[:, k, :],
                                             start=(k == 0), stop=(k == HK - 1))
                        sg = sm.tile([P, IQ], F32, tag="sg", name=f"sg{e}_{q}_{t}")
                        nc.scalar.activation(out=sg[:], in_=hgu[:, :IQ],
                                             func=ACT.Silu)
                        hq = sm.tile([P, IQ], F32R, tag="hq", name=f"hq{e}_{q}_{t}")
                        nc.vector.tensor_tensor(out=hq[:], in0=sg[:],
                                                in1=hgu[:, IQ:], op=ALU.mult)
                        # transpose the two 128-chunks of this quarter into hts
                        for j in range(IQ // P):
                            tp = pstp.tile([P, P], F32R, tag="tp",
                                           name=f"tph{e}_{q}_{t}_{j}")
                            nc.tensor.transpose(out=tp[:],
                                                in_=hq[:, j * P:(j + 1) * P],
                                                identity=ident_r[:])
                            nc.vector.tensor_copy(
                                out=hts[t][:, q * (IQ // P) + j, :], in_=tp[:])

                # pass B: down-projection; wd streamed in 512-wide column chunks
                for qc in range(4):
                    csl = slice(qc * 512, (qc + 1) * 512)  # Y column slice
                    wdt = wdp.tile([P, IK, 512], F32R, tag="wd", name=f"wd{e}_{qc}")
                    nc.sync.dma_start(out=wdt[:], in_=wd_d[e, qc])
                    for t in range(ntiles):
                        yps = psY.tile([P, 512], F32, tag="y", name=f"y{e}_{qc}_{t}")
                        for k in range(IK):
                            nc.tensor.matmul(
                                out=yps[:], lhsT=hts[t][:, k, :],
                                rhs=wdt[:, k, :],
                                start=(k == 0), stop=(k == IK - 1))
                        ysb = sm.tile([P, 512], F32, tag="ysb",
                                      name=f"ysb{e}_{qc}_{t}")
                        nc.vector.tensor_copy(out=ysb[:], in_=yps[:])
                        row0 = bases[e] + t * P
                        nc.sync.dma_start(out=Y_d[row0:row0 + P, csl], in_=ysb[:])

            # combine: per owned token, weighted sum of its two expert rows
            for ct in range(TPC // P):
                pp = posw_all[:, ct, :]
                wtw = ww_all[:, ct, :]
                ya = xtp.tile([P, H], F32, tag="xt0", name=f"ya{ct}")
                nc.gpsimd.indirect_dma_start(
                    out=ya[:], out_offset=None, in_=Y_d[:],
                    in_offset=bass.IndirectOffsetOnAxis(ap=pp[:, 0:1], axis=0))
                yb = xtp.tile([P, H], F32, tag="xt1", name=f"yb{ct}")
                nc.gpsimd.indirect_dma_start(
                    out=yb[:], out_offset=None, in_=Y_d[:],
                    in_offset=bass.IndirectOffsetOnAxis(ap=pp[:, 1:2], axis=0))
                tmpa = xtp.tile([P, H], F32, tag="xt2", name=f"tmpa{ct}")
                nc.vector.tensor_scalar_mul(tmpa[:], ya[:], wtw[:, 0:1])
                acc = xtp.tile([P, H], F32, tag="xt3", name=f"acc{ct}")
                nc.vector.scalar_tensor_tensor(
                    out=acc[:], in0=yb[:], scalar=wtw[:, 1:2], in1=tmpa[:],
                    op0=ALU.mult, op1=ALU.add)
                nc.sync.dma_start(out=y_d[ct * P:(ct + 1) * P, :], in_=acc[:])

    nc.finalize()
    return nc


# --------------------------------------------------------------------------
# Host orchestration
# --------------------------------------------------------------------------
def _route_host(probs):
    """Index bookkeeping only. probs: [T, 8] f32 from device.

    Returns per-core dicts of int32/f32 side inputs plus the cap tuple."""
    per_core = []
    cnt = np.zeros((NCORES, E), np.int64)
    sel_all = []
    for c in range(NCORES):
        pl = probs[c * TPC:(c + 1) * TPC]
        # stable argsort of -p == lax.top_k tie semantics (lowest index first)
        top2 = np.argsort(-pl, axis=1, kind="stable")[:, :2]
        sel_all.append(top2)
        for e in range(E):
            cnt[c, e] = int(((top2 == e).any(axis=1)).sum())
    caps = tuple(int(P * np.ceil(cnt[:, e].max() / P)) for e in range(E))
    bases = np.cumsum([0] + list(caps))[:-1]
    nb = int(sum(caps))

    for c in range(NCORES):
        pl = probs[c * TPC:(c + 1) * TPC]
        top2 = sel_all[c]
        bidx = np.zeros(nb, np.int32)
        posmap = np.zeros((TPC, E), np.int64)  # position of token t in bucket e
        for e in range(E):
            rows = np.nonzero((top2 == e).any(axis=1))[0]
            bidx[bases[e]:bases[e] + len(rows)] = rows.astype(np.int32)
            posmap[rows, e] = bases[e] + np.arange(len(rows))
        tok = np.arange(TPC)
        posw = np.stack([posmap[tok, top2[:, 0]], posmap[tok, top2[:, 1]]],
                        axis=1).astype(np.int32)
        ww = np.stack([pl[tok, top2[:, 0]], pl[tok, top2[:, 1]]],
                      axis=1).astype(np.float32)
        per_core.append({"bidx": bidx, "posw": np.ascontiguousarray(posw),
                         "ww": np.ascontiguousarray(ww)})
    return per_core, caps


def kernel(x, gate_w, wg, wu, wd):
    x = np.ascontiguousarray(x, dtype=np.float32)
    gate_w = np.ascontiguousarray(gate_w, dtype=np.float32)
    wg = np.ascontiguousarray(wg, dtype=np.float32)
    wu = np.ascontiguousarray(wu, dtype=np.float32)
    wd = np.ascontiguousarray(wd, dtype=np.float32)

    xt = x.reshape(T, H)
    gwt = np.ascontiguousarray(gate_w.T)

    # launch 1: router
    r_nc = _router_nc()
    in_maps1 = [{"xs": xt[c * TPC:(c + 1) * TPC], "gwt": gwt} for c in CORE_IDS]
    res1 = run_bass_kernel_spmd(r_nc, in_maps1, CORE_IDS).results
    logits = np.concatenate([res1[c]["logits"] for c in CORE_IDS], axis=0)
    probs = np.concatenate([res1[c]["probs"] for c in CORE_IDS], axis=0)

    # host: index bookkeeping
    side, caps = _route_host(probs)
    global _LAST_CAPS
    _LAST_CAPS = caps

    # launch 2: MoE (weights pre-staged into per-partition-contiguous layout)
    wgr = wg.reshape(E, HK, P, NQ, IQ).transpose(0, 3, 2, 1, 4)
    wur = wu.reshape(E, HK, P, NQ, IQ).transpose(0, 3, 2, 1, 4)
    wgur = np.ascontiguousarray(np.concatenate([wgr, wur], axis=4))
    wdr = np.ascontiguousarray(
        wd.reshape(E, IK, P, 4, 512).transpose(0, 3, 2, 1, 4))
    m_nc = _moe_nc(caps)
    in_maps2 = []
    for c in CORE_IDS:
        m = {"xs": xt[c * TPC:(c + 1) * TPC], "wgu": wgur, "wd": wdr}
        m.update(side[c])
        in_maps2.append(m)
    res2 = run_bass_kernel_spmd(m_nc, in_maps2, CORE_IDS).results
    out = np.concatenate([res2[c]["y"] for c in CORE_IDS], axis=0)
    return out.reshape(B, S, H), logits


# revision 35
# speedup vs baseline: 1.3328x; 1.0206x over previous
"""Trainium2 Bass kernel for an 8-expert top-2 MoE (nn_BaseMoe).

Strategy (data-parallel routed, 2 launches, all FLOPs on device):
  - 8 cores; core c owns tokens [c*2048, (c+1)*2048).
  - Launch 1 (router): per core, logits = x_shard @ gate_w.T (fp32 PE matmul)
    and probs = softmax(logits) on device. Host only does index bookkeeping:
    stable top-2 selection, per-(core, expert) token buckets, bucket
    positions for the final combine.
  - Launch 2 (MoE): per core, for each expert e: indirect-DMA gather the
    bucket's token rows, SwiGLU MLP in float32r (1 cyc/row on PE), write
    rows to an internal DRAM buffer Y. Finally, per owned token, gather its
    two expert rows from Y and do the weighted sum on device.

The kernel returns (out [4,4096,2048] f32, router_logits [16384,8] f32),
matching the reference's tuple.
"""

import sys
import functools

sys.path.insert(0, "/opt/trn_rl_repo")

import numpy as np

import concourse.bass as bass
import concourse.mybir as mybir
from concourse import bacc
from concourse.tile import TileContext
from concourse.bass_utils import run_bass_kernel_spmd
from concourse.masks import make_identity

F32 = mybir.dt.float32
F32R = mybir.dt.float32r
I32 = mybir.dt.int32
AX = mybir.AxisListType
ALU = mybir.AluOpType
ACT = mybir.ActivationFunctionType

B, S, H, I, E, TOPK = 4, 4096, 2048, 1024, 8, 2
T = B * S            # 16384 tokens
NCORES = 8
TPC = T // NCORES    # 2048 tokens per core
P = 128              # partitions
HK = H // P          # 16 contraction chunks over H
IK = I // P          # 8 contraction chunks over I
NQ = 4               # I split into 4 quarters for gate/up weight streaming
IQ = I // NQ         # 256

CORE_IDS = list(range(NCORES))
_LAST_CAPS = None


# --------------------------------------------------------------------------
# Launch 1: router
# --------------------------------------------------------------------------
@functools.lru_cache(maxsize=1)
def _router_nc():
    nc = bacc.Bacc()
    xs_d = nc.declare_dram_parameter("xs", [TPC, H], F32, isOutput=False)
    gwt_d = nc.declare_dram_parameter("gwt", [H, E], F32, isOutput=False)
    logits_d = nc.declare_dram_parameter("logits", [TPC, E], F32, isOutput=True)
    probs_d = nc.declare_dram_parameter("probs", [TPC, E], F32, isOutput=True)

    with TileContext(nc) as tc:
        with tc.tile_pool(name="const", bufs=1) as cpool, \
             tc.tile_pool(name="sb", bufs=3) as sb, \
             tc.tile_pool(name="ps", bufs=2, space="PSUM") as ps:
            ident = cpool.tile([P, P], F32)
            make_identity(nc, ident[:])
            gwt_sb = cpool.tile([P, HK, E], F32)
            nc.sync.dma_start(
                out=gwt_sb[:], in_=gwt_d[:].rearrange("(k p) e -> p k e", p=P))

            for ti in range(TPC // P):
                xg = sb.tile([P, H], F32, tag="xg")
                nc.sync.dma_start(out=xg[:], in_=xs_d[ti * P:(ti + 1) * P, :])
                lg_ps = ps.tile([P, E], F32, tag="lg")
                for k in range(HK):
                    tp = ps.tile([P, P], F32, tag="tp")
                    nc.tensor.transpose(
                        out=tp[:], in_=xg[:, k * P:(k + 1) * P], identity=ident[:])
                    xtk = sb.tile([P, P], F32, tag="xtk")
                    nc.vector.tensor_copy(out=xtk[:], in_=tp[:])
                    nc.tensor.matmul(out=lg_ps[:], lhsT=xtk[:], rhs=gwt_sb[:, k, :],
                                     start=(k == 0), stop=(k == HK - 1))
                lg = sb.tile([P, E], F32, tag="lgsb")
                nc.vector.tensor_copy(out=lg[:], in_=lg_ps[:])
                nc.sync.dma_start(out=logits_d[ti * P:(ti + 1) * P, :], in_=lg[:])

                # softmax over the 8 experts (free dim)
                rmax = sb.tile([P, 1], F32, tag="rmax")
                nc.vector.reduce_max(rmax[:], lg[:], axis=AX.X)
                negmax = sb.tile([P, 1], F32, tag="negmax")
                nc.vector.tensor_scalar_mul(negmax[:], rmax[:], -1.0)
                pex = sb.tile([P, E], F32, tag="pex")
                nc.scalar.activation(out=pex[:], in_=lg[:], func=ACT.Exp,
                                     bias=negmax[:, :1], scale=1.0)
                ssum = sb.tile([P, 1], F32, tag="ssum")
                nc.vector.reduce_sum(ssum[:], pex[:], axis=AX.X)
                rs = sb.tile([P, 1], F32, tag="rs")
                nc.vector.reciprocal(rs[:], ssum[:])
                probs = sb.tile([P, E], F32, tag="probs")
                nc.vector.tensor_scalar_mul(probs[:], pex[:], rs[:, :1])
                nc.sync.dma_start(out=probs_d[ti * P:(ti + 1) * P, :], in_=probs[:])

    nc.finalize()
    return nc


# --------------------------------------------------------------------------
# Launch 2: bucketed expert MLP + on-device combine
# --------------------------------------------------------------------------
@functools.lru_cache(maxsize=4)
def _moe_nc(caps, nreal=None):
    """caps: 8 per-expert bucket capacities (multiples of 128); nreal: true
    max row count per expert (pads beyond it are neither gathered, computed
    at full width, nor written)."""
    caps = tuple(int(c) for c in caps)
    if nreal is None:
        nreal = caps
    nreal = tuple(int(n) for n in nreal)
    nb = sum(caps)
    bases = np.cumsum([0] + list(caps))[:-1]

    nc = bacc.Bacc()
    xs_d = nc.declare_dram_parameter("xs", [TPC, H], F32R, isOutput=False)
    wgu_d = nc.declare_dram_parameter("wgu", [E, NQ, P, HK, 2 * IQ], F32R,
                                      isOutput=False)
    wd_d = nc.declare_dram_parameter("wd", [E, 4, P, IK, 512], F32R, isOutput=False)
    bidx_d = nc.declare_dram_parameter("bidx", [nb], I32, isOutput=False)
    posw_d = nc.declare_dram_parameter("posw", [TPC, 2], I32, isOutput=False)
    ww_d = nc.declare_dram_parameter("ww", [TPC, 2], F32, isOutput=False)
    y_d = nc.declare_dram_parameter("y", [TPC, H], F32, isOutput=True)
    Y_d = nc.dram_tensor("Ybuf", [nb, H], F32)

    nflat = nb // P

    with TileContext(nc) as tc:
        with tc.tile_pool(name="const", bufs=1) as cpool, \
             tc.tile_pool(name="wab", bufs=2) as wab, \
             tc.tile_pool(name="wdp", bufs=2) as wdp, \
             tc.tile_pool(name="xt", bufs=1) as xtp, \
             tc.tile_pool(name="hb", bufs=1) as hbp, \
             tc.tile_pool(name="sb", bufs=2) as sb, \
             tc.tile_pool(name="sm", bufs=2) as sm, \
             tc.tile_pool(name="psA", bufs=3, space="PSUM") as psA, \
             tc.tile_pool(name="pstp", bufs=3, space="PSUM") as pstp, \
             tc.tile_pool(name="psY", bufs=2, space="PSUM") as psY:
            ident = sm.tile([P, P], F32, tag="small")
            make_identity(nc, ident[:])
            ident_r = cpool.tile([P, P], F32R)
            nc.vector.tensor_copy(out=ident_r[:], in_=ident[:])
            idx_all = cpool.tile([P, nflat], I32)
            nc.sync.dma_start(
                out=idx_all[:], in_=bidx_d[:].rearrange("(n p) -> p n", p=P))
            posw_all = cpool.tile([P, TPC // P, 2], I32)
            nc.sync.dma_start(
                out=posw_all[:],
                in_=posw_d[:].rearrange("(n p) o -> p n o", p=P))
            ww_all = cpool.tile([P, TPC // P, 2], F32)
            nc.sync.dma_start(
                out=ww_all[:], in_=ww_d[:].rearrange("(n p) o -> p n o", p=P))

            for e in range(E):
                ntiles = caps[e] // P
                xts = []
                hts = []
                # gather + transpose this expert's bucket rows
                rows_of = [min(P, 32 * ((nreal[e] - t * P + 31) // 32))
                           for t in range(ntiles)]
                for t in range(ntiles):
                    rws = rows_of[t]
                    flat = (bases[e] + t * P) // P
                    xg = sb.tile([P, H], F32R, tag="xg", name=f"xg{e}_{t}")
                    nc.gpsimd.indirect_dma_start(
                        out=xg[:rws, :], out_offset=None, in_=xs_d[:],
                        in_offset=bass.IndirectOffsetOnAxis(
                            ap=idx_all[:rws, flat:flat + 1], axis=0))
                    xt = xtp.tile([P, HK, P], F32R, tag=f"xt{t}", name=f"xt{t}_{e}")
                    for k in range(HK):
                        tp = pstp.tile([P, P], F32R, tag="tp", name=f"tpx{e}_{t}_{k}")
                        nc.tensor.transpose(
                            out=tp[:, :rws], in_=xg[:rws, k * P:(k + 1) * P],
                            identity=ident_r[:rws, :rws])
                        nc.vector.tensor_copy(out=xt[:, k, :rws], in_=tp[:, :rws])
                    xts.append(xt)
                    ht = hbp.tile([P, IK, P], F32R, tag=f"h{t}", name=f"h{t}_{e}")
                    hts.append(ht)

                # pass A: gate/up in I quarters (weights streamed, double-buffered)
                # h is stored pre-transposed: hts[t][:, c, :] = h[:, c*128:+128].T
                for q in range(NQ):
                    qsl = slice(q * IQ, (q + 1) * IQ)
                    wgq = wab.tile([P, HK, 2 * IQ], F32R, tag="wgq",
                                   name=f"wgq{e}_{q}")
                    nc.sync.dma_start(out=wgq[:], in_=wgu_d[e, q])
                    for t in range(ntiles):
                        rws = rows_of[t]
                        hgu = psA.tile([P, 2 * IQ], F32, tag="hgu",
                                       name=f"hgu{e}_{q}_{t}")
                        for k in range(HK):
                            nc.tensor.matmul(out=hgu[:rws, :],
                                             lhsT=xts[t][:, k, :rws],
                                             rhs=wgq[:, k, :],
                                             start=(k == 0), stop=(k == HK - 1))
                        sg = sm.tile([P, IQ], F32, tag="sg", name=f"sg{e}_{q}_{t}")
                        nc.scalar.activation(out=sg[:rws, :], in_=hgu[:rws, :IQ],
                                             func=ACT.Silu)
                        hq = sm.tile([P, IQ], F32R, tag="hq", name=f"hq{e}_{q}_{t}")
                        nc.vector.tensor_tensor(out=hq[:rws, :], in0=sg[:rws, :],
                                                in1=hgu[:rws, IQ:], op=ALU.mult)
                        # transpose the two 128-chunks of this quarter into hts
                        for j in range(IQ // P):
                            tp = pstp.tile([P, P], F32R, tag="tp",
                                           name=f"tph{e}_{q}_{t}_{j}")
                            nc.tensor.transpose(out=tp[:, :rws],
                                                in_=hq[:rws, j * P:(j + 1) * P],
                                                identity=ident_r[:rws, :rws])
                            nc.vector.tensor_copy(
                                out=hts[t][:, q * (IQ // P) + j, :rws],
                                in_=tp[:, :rws])

                # pass B: down-projection; wd streamed in 512-wide column chunks
                for qc in range(4):
                    csl = slice(qc * 512, (qc + 1) * 512)  # Y column slice
                    wdt = wdp.tile([P, IK, 512], F32R, tag="wd", name=f"wd{e}_{qc}")
                    nc.sync.dma_start(out=wdt[:], in_=wd_d[e, qc])
                    for t in range(ntiles):
                        rws = rows_of[t]
                        yps = psY.tile([P, 512], F32, tag="y", name=f"y{e}_{qc}_{t}")
                        for k in range(IK):
                            nc.tensor.matmul(
                                out=yps[:rws, :], lhsT=hts[t][:, k, :rws],
                                rhs=wdt[:, k, :],
                                start=(k == 0), stop=(k == IK - 1))
                        ysb = sm.tile([P, 512], F32, tag="ysb",
                                      name=f"ysb{e}_{qc}_{t}")
                        nc.vector.tensor_copy(out=ysb[:rws, :], in_=yps[:rws, :])
                        row0 = bases[e] + t * P
                        nc.sync.dma_start(out=Y_d[row0:row0 + rws, csl],
                                          in_=ysb[:rws, :])

            # combine: per owned token, weighted sum of its two expert rows
            for ct in range(TPC // P):
                pp = posw_all[:, ct, :]
                wtw = ww_all[:, ct, :]
                ya = xtp.tile([P, H], F32, tag="xt0", name=f"ya{ct}")
                nc.gpsimd.indirect_dma_start(
                    out=ya[:], out_offset=None, in_=Y_d[:],
                    in_offset=bass.IndirectOffsetOnAxis(ap=pp[:, 0:1], axis=0))
                yb = xtp.tile([P, H], F32, tag="xt1", name=f"yb{ct}")
                nc.gpsimd.indirect_dma_start(
                    out=yb[:], out_offset=None, in_=Y_d[:],
                    in_offset=bass.IndirectOffsetOnAxis(ap=pp[:, 1:2], axis=0))
                tmpa = xtp.tile([P, H], F32, tag="xt2", name=f"tmpa{ct}")
                nc.vector.tensor_scalar_mul(tmpa[:], ya[:], wtw[:, 0:1])
                acc = xtp.tile([P, H], F32, tag="xt3", name=f"acc{ct}")
                nc.vector.scalar_tensor_tensor(
                    out=acc[:], in0=yb[:], scalar=wtw[:, 1:2], in1=tmpa[:],
                    op0=ALU.mult, op1=ALU.add)
                nc.sync.dma_start(out=y_d[ct * P:(ct + 1) * P, :], in_=acc[:])

    nc.finalize()
    return nc


# --------------------------------------------------------------------------
# Host orchestration
# --------------------------------------------------------------------------
def _route_host(probs):
    """Index bookkeeping only. probs: [T, 8] f32 from device.

    Returns per-core dicts of int32/f32 side inputs plus the cap tuple."""
    per_core = []
    cnt = np.zeros((NCORES, E), np.int64)
    sel_all = []
    for c in range(NCORES):
        pl = probs[c * TPC:(c + 1) * TPC]
        # stable argsort of -p == lax.top_k tie semantics (lowest index first)
        top2 = np.argsort(-pl, axis=1, kind="stable")[:, :2]
        sel_all.append(top2)
        for e in range(E):
            cnt[c, e] = int(((top2 == e).any(axis=1)).sum())
    caps = tuple(int(P * np.ceil(cnt[:, e].max() / P)) for e in range(E))
    nreal = tuple(int(cnt[:, e].max()) for e in range(E))
    bases = np.cumsum([0] + list(caps))[:-1]
    nb = int(sum(caps))

    for c in range(NCORES):
        pl = probs[c * TPC:(c + 1) * TPC]
        top2 = sel_all[c]
        bidx = np.zeros(nb, np.int32)
        posmap = np.zeros((TPC, E), np.int64)  # position of token t in bucket e
        for e in range(E):
            rows = np.nonzero((top2 == e).any(axis=1))[0]
            bidx[bases[e]:bases[e] + len(rows)] = rows.astype(np.int32)
            posmap[rows, e] = bases[e] + np.arange(len(rows))
        tok = np.arange(TPC)
        posw = np.stack([posmap[tok, top2[:, 0]], posmap[tok, top2[:, 1]]],
                        axis=1).astype(np.int32)
        ww = np.stack([pl[tok, top2[:, 0]], pl[tok, top2[:, 1]]],
                      axis=1).astype(np.float32)
        per_core.append({"bidx": bidx, "posw": np.ascontiguousarray(posw),
                         "ww": np.ascontiguousarray(ww)})
    return per_core, caps, nreal


def kernel(x, gate_w, wg, wu, wd):
    x = np.ascontiguousarray(x, dtype=np.float32)
    gate_w = np.ascontiguousarray(gate_w, dtype=np.float32)
    wg = np.ascontiguousarray(wg, dtype=np.float32)
    wu = np.ascontiguousarray(wu, dtype=np.float32)
    wd = np.ascontiguousarray(wd, dtype=np.float32)

    xt = x.reshape(T, H)
    gwt = np.ascontiguousarray(gate_w.T)

    # launch 1: router
    r_nc = _router_nc()
    in_maps1 = [{"xs": xt[c * TPC:(c + 1) * TPC], "gwt": gwt} for c in CORE_IDS]
    res1 = run_bass_kernel_spmd(r_nc, in_maps1, CORE_IDS).results
    logits = np.concatenate([res1[c]["logits"] for c in CORE_IDS], axis=0)
    probs = np.concatenate([res1[c]["probs"] for c in CORE_IDS], axis=0)

    # host: index bookkeeping
    side, caps, nreal = _route_host(probs)
    global _LAST_CAPS
    _LAST_CAPS = (caps, nreal)

    # launch 2: MoE (weights pre-staged into per-partition-contiguous layout)
    wgr = wg.reshape(E, HK, P, NQ, IQ).transpose(0, 3, 2, 1, 4)
    wur = wu.reshape(E, HK, P, NQ, IQ).transpose(0, 3, 2, 1, 4)
    wgur = np.ascontiguousarray(np.concatenate([wgr, wur], axis=4))
    wdr = np.ascontiguousarray(
        wd.reshape(E, IK, P, 4, 512).transpose(0, 3, 2, 1, 4))
    m_nc = _moe_nc(caps, nreal)
    in_maps2 = []
    for c in CORE_IDS:
        m = {"xs": xt[c * TPC:(c + 1) * TPC], "wgu": wgur, "wd": wdr}
        m.update(side[c])
        in_maps2.append(m)
    res2 = run_bass_kernel_spmd(m_nc, in_maps2, CORE_IDS).results
    out = np.concatenate([res2[c]["y"] for c in CORE_IDS], axis=0)
    return out.reshape(B, S, H), logits
